# revision 1
# baseline (speedup 1.0000x reference)
"""CapsuleLayer dynamic-routing kernel for 8x trn2 NeuronCores.

Sharding: input-capsule axis i (2048) split 8 ways (256/core). Routing
softmax over j is core-local; the per-iteration s = sum_i c*u_hat is
partial per core and finished with an 8-core AllReduce. u_hat is never
materialized: each routing pass recomputes the needed contractions from
W directly (contraction over (i,l) or k), keeping PE efficiency high.

Host-side layouts per core (i-shard, IS=256, IL=IS*L=4096):
  XT [IL, B]        x^T               (pass-0 stationary / Y-formation)
  XB [B, IL]        x                 (g-pass elementwise)
  WB [IL, J*K]      W as [(i,l),(j,k)] (pass-0 + s-pass moving)
  W2 [J, K, IL]     W as [j,k,(i,l)]  (t-pass moving)
"""

import numpy as np

B, I_FULL, L = 64, 2048, 16
J, K = 64, 32
NCORES = 8
IS = I_FULL // NCORES          # 256 i per core
IL = IS * L                    # 4096
NCH = IL // 128                # 32 contraction chunks of 128
JK = J * K                     # 2048
EPS = 1e-7
ROUTINGS = 3

_cache = {}
_NO_CC = False


def _build():
    import concourse.bass as bass
    import concourse.bacc as bacc
    import concourse.mybir as mybir
    import concourse.tile as tile

    f32 = mybir.dt.float32
    bf16 = mybir.dt.bfloat16

    nc = bacc.Bacc("TRN2", target_bir_lowering=False, debug=False)

    XT_d = nc.dram_tensor("xt", [IL, B], bf16, kind="ExternalInput")
    XB_d = nc.dram_tensor("xb", [B, IL], bf16, kind="ExternalInput")
    WB_d = nc.dram_tensor("wb", [IL, JK], bf16, kind="ExternalInput")
    W2_d = nc.dram_tensor("w2", [J, K, IL], bf16, kind="ExternalInput")
    out_d = nc.dram_tensor("v_out", [B, JK], f32, kind="ExternalOutput")

    with tile.TileContext(nc) as tc:
        with (
            tc.tile_pool(name="res", bufs=1) as res,
            tc.tile_pool(name="wbs", bufs=2) as wbs,
            tc.tile_pool(name="w2s", bufs=2) as w2s,
            tc.tile_pool(name="crep", bufs=2) as crepp,
            tc.tile_pool(name="yp", bufs=2) as yp,
            tc.tile_pool(name="prod", bufs=1) as prodp,
            tc.tile_pool(name="ps", bufs=1, space="PSUM") as ps,
            tc.tile_pool(name="ptp", bufs=4, space="PSUM") as ptp,
            tc.tile_pool(name="dram", bufs=2, space="DRAM") as dram,
        ):
            # ---- resident SBUF tensors ----
            XT_s = res.tile([128, NCH * B], bf16)      # chunk-major x^T
            XB_s = res.tile([B, IL], bf16)
            G_s = res.tile([B, J * IS], f32)           # routing logits [b,(j,i)]
            E_s = res.tile([B, J * IS], bf16)          # exp(logits)
            esum_s = res.tile([B, IS], f32)
            R_s = res.tile([B, IS], f32)
            s_loc = res.tile([B, JK], f32)
            s_full = res.tile([B, JK], f32)
            V_cum = res.tile([B, JK], f32)
            V_bf = res.tile([B, JK], bf16)
            V_T4 = res.tile([128, B * J], bf16)        # 4 stacked [k, b*64+j]
            sq = res.tile([B, J], f32)
            d1 = res.tile([B, J], f32)
            d2 = res.tile([B, J], f32)
            d3 = res.tile([B, J], f32)
            rr = res.tile([B, J], f32)
            sc = res.tile([B, J], f32)

            # load residents
            nc.sync.dma_start(
                XT_s[:, :].rearrange("p (c b) -> p c b", b=B),
                XT_d.ap().rearrange("(c p) b -> p c b", p=128),
            )
            nc.sync.dma_start(XB_s[:, :], XB_d[:, :])

            cc_in = dram.tile([B, JK], f32)
            cc_out = dram.tile([B, JK], f32)
            vbuf = dram.tile([B, JK], bf16)
            cbuf = dram.tile([B, J * IS], bf16)

            def all_reduce_s():
                if _NO_CC:
                    nc.vector.tensor_copy(s_full[:, :], s_loc[:, :])
                    return
                nc.sync.dma_start(cc_in[:, :], s_loc[:, :])
                nc.gpsimd.collective_compute(
                    "AllReduce",
                    mybir.AluOpType.add,
                    replica_groups=[list(range(NCORES))],
                    ins=[cc_in.opt()],
                    outs=[cc_out.opt()],
                )
                nc.sync.dma_start(s_full[:, :], cc_out[:, :])

            def squash_and_accum(first):
                # v = s*sq/(1+sq)/sqrt(sq+eps); V_cum += v (v left in s_full)
                nc.scalar.square(s_loc[:, :], s_full[:, :])
                nc.vector.tensor_reduce(
                    sq[:, :],
                    s_loc[:, :].rearrange("b (j k) -> b j k", k=K),
                    axis=mybir.AxisListType.X,
                    op=mybir.AluOpType.add,
                )
                nc.vector.tensor_scalar_add(d1[:, :], sq[:, :], EPS)
                nc.scalar.sqrt(d1[:, :], d1[:, :])
                nc.vector.tensor_scalar_add(d2[:, :], sq[:, :], 1.0)
                nc.vector.tensor_mul(d3[:, :], d1[:, :], d2[:, :])
                nc.vector.reciprocal(rr[:, :], d3[:, :])
                nc.vector.tensor_mul(sc[:, :], sq[:, :], rr[:, :])
                nc.vector.tensor_tensor(
                    out=s_full[:, :],
                    in0=s_full[:, :],
                    in1=sc[:, :].rearrange("b (j o) -> b j o", o=1).broadcast_to([B, J, K]),
                    op=mybir.AluOpType.mult,
                )
                if first:
                    nc.vector.tensor_copy(V_cum[:, :], s_full[:, :])
                else:
                    nc.vector.tensor_add(V_cum[:, :], V_cum[:, :], s_full[:, :])
                # V_bf stored k-major: V_bf[b, k*64+j] = V_cum[b, j*32+k]
                nc.vector.tensor_copy(
                    V_bf[:, :],
                    V_cum[:, :].rearrange("b (j k) -> b k j", k=K),
                )
                # V_T[k, b*64+j] = V_bf[b, k*64+j], via DRAM bounce
                nc.sync.dma_start(vbuf[:, :], V_bf[:, :])
                for t in range(4):
                    nc.sync.dma_start(
                        V_T4[t * K:(t + 1) * K, :].rearrange(
                            "k (b j) -> k b j", j=J
                        ),
                        vbuf[:, :].rearrange("b (k j) -> k b j", k=K),
                    )

            # ======== iteration 0: c = 1/J ========
            ps0 = ps.tile([128, JK], f32, tag="ps")
            for ch in range(NCH):
                wbt = wbs.tile([128, JK], bf16)
                nc.sync.dma_start(wbt[:, :], WB_d[ch * 128:(ch + 1) * 128, :])
                for q in range(4):
                    nc.tensor.matmul(
                        ps0[:B, q * 512:(q + 1) * 512],
                        XT_s[:, ch * B:(ch + 1) * B],
                        wbt[:, q * 512:(q + 1) * 512],
                        start=(ch == 0),
                        stop=(ch == NCH - 1),
                    )
            nc.scalar.mul(s_loc[:, :], ps0[:B, :], 1.0 / J)
            all_reduce_s()
            squash_and_accum(first=True)

            # ======== iterations 1..2 ========
            for r in range(1, ROUTINGS):
                # ---- t-pass + g: logits G[b,(j,i)] = sum_k V.u_hat ----
                # 4 j's row-packed via tile_position: each 32-row strip of
                # the PE array runs an independent k=32-contraction matmul
                for jg in range(J // 4):
                    w2t = w2s.tile([128, IL], bf16, tag="w2t")
                    nc.sync.dma_start(
                        w2t[:, :],
                        W2_d[jg * 4:(jg + 1) * 4, :, :].rearrange(
                            "a k il -> (a k) il"
                        ),
                    )
                    for q in range(8):
                        pts = []
                        for t in range(4):
                            j = jg * 4 + t
                            pt = ptp.tile([B, 512], f32, tag="pt")
                            nc.tensor.matmul(
                                pt[:, :],
                                V_T4[t * K:(t + 1) * K, :].rearrange(
                                    "k (b j) -> k b j", j=J
                                )[:, :, j],
                                w2t[t * K:(t + 1) * K,
                                    q * 512:(q + 1) * 512],
                                start=True,
                                stop=True,
                                tile_position=(t * K, 0),
                            )
                            pts.append(pt)
                        for t in range(4):
                            j = jg * 4 + t
                            prod = prodp.tile([B, 512], f32, tag="prod")
                            nc.vector.tensor_tensor(
                                out=prod[:, :],
                                in0=pts[t][:, :],
                                in1=XB_s[:, q * 512:(q + 1) * 512],
                                op=mybir.AluOpType.mult,
                            )
                            nc.vector.tensor_reduce(
                                G_s[:, j * IS + q * 32: j * IS + (q + 1) * 32],
                                prod[:, :].rearrange("b (i l) -> b i l", l=L),
                                axis=mybir.AxisListType.X,
                                op=mybir.AluOpType.add,
                            )
                # ---- softmax over j ----
                nc.scalar.activation(
                    G_s[:, :], G_s[:, :], mybir.ActivationFunctionType.Exp
                )
                nc.vector.tensor_reduce(
                    esum_s[:, :],
                    G_s[:, :].rearrange("b (j i) -> b i j", j=J),
                    axis=mybir.AxisListType.X,
                    op=mybir.AluOpType.add,
                )
                nc.vector.reciprocal(R_s[:, :], esum_s[:, :])
                nc.vector.tensor_tensor(
                    out=E_s[:, :],
                    in0=G_s[:, :],
                    in1=R_s[:, :]
                    .rearrange("b (o i) -> b o i", o=1)
                    .broadcast_to([B, J, IS]),
                    op=mybir.AluOpType.mult,
                )
                # ---- s-pass: s[b,(j,k)] = sum_il (c*x) . WB ----
                # rows are l-major: chunk ch = (l=ch>>1, i-half=ch&1)
                nc.sync.dma_start(cbuf[:, :], E_s[:, :])
                creps = []
                for ci in range(2):
                    crep = crepp.tile([128, B * J], bf16, tag="crep")
                    nc.sync.dma_start(
                        crep[:, :].rearrange("p (b j) -> p b j", j=J),
                        cbuf[:, :].rearrange("b (j i) -> i b j", j=J)[
                            ci * 128:(ci + 1) * 128
                        ],
                    )
                    creps.append(crep)
                pss = ps.tile([128, JK], f32, tag="ps")
                for ch in range(NCH):
                    # write Y in (j,b) order so s-matmul stationaries are
                    # contiguous; inputs keep their (b,j)/broadcast layouts
                    yt = yp.tile([128, J * B], bf16)
                    nc.vector.tensor_tensor(
                        out=yt[:, :].rearrange("p (j b) -> p j b", j=J),
                        in0=creps[ch % 2][:, :].rearrange(
                            "p (b j) -> p j b", j=J
                        ),
                        in1=XT_s[:, ch * B:(ch + 1) * B]
                        .rearrange("p (o b) -> p o b", o=1)
                        .broadcast_to([128, J, B]),
                        op=mybir.AluOpType.mult,
                    )
                    wbt = wbs.tile([128, JK], bf16)
                    nc.sync.dma_start(
                        wbt[:, :], WB_d[ch * 128:(ch + 1) * 128, :]
                    )
                    for j in range(J):
                        # one start=True per PSUM bank (16 j-slices/bank):
                        # its bank-wide has_written clear must precede all
                        # other j's writes, which land with start=False
                        nc.tensor.matmul(
                            pss[:B, j * K:(j + 1) * K],
                            yt[:, j * B:(j + 1) * B],
                            wbt[:, j * K:(j + 1) * K],
                            start=(ch == 0 and j % 16 == 0),
                            stop=(ch == NCH - 1),
                            skip_group_check=True,
                        )
                nc.vector.tensor_copy(s_loc[:, :], pss[:B, :])
                all_reduce_s()
                squash_and_accum(first=False)

            # s_full now holds v_2 = output
            nc.sync.dma_start(out_d[:, :], s_full[:, :])

    nc.finalize()
    return nc


def kernel(inputs, W):
    from concourse.bass_utils import run_bass_kernel_spmd

    if "nc" not in _cache:
        _cache["nc"] = _build()
    nc = _cache["nc"]

    in_maps = []
    for c in range(NCORES):
        xs = inputs[:, c * IS:(c + 1) * IS, :].astype(np.float32)
        Ws = W[:, c * IS:(c + 1) * IS, :, :].astype(np.float32)
        import ml_dtypes

        bf = ml_dtypes.bfloat16
        XT = np.ascontiguousarray(
            xs.transpose(2, 1, 0).reshape(IL, B)
        ).astype(bf)
        XB = np.ascontiguousarray(xs.reshape(B, IL)).astype(bf)
        WB = np.ascontiguousarray(
            Ws.transpose(3, 1, 0, 2).reshape(IL, JK)
        ).astype(bf)
        W2 = np.ascontiguousarray(
            Ws.transpose(0, 2, 1, 3).reshape(J, K, IL)
        ).astype(bf)
        in_maps.append({"xt": XT, "xb": XB, "wb": WB, "w2": W2})

    _cache["in_maps"] = in_maps
    globals()["_last_in_maps"] = in_maps
    res = run_bass_kernel_spmd(nc, in_maps, core_ids=list(range(NCORES)))
    v = res.results[0]["v_out"]
    return np.asarray(v, dtype=np.float32).reshape(B, J, K)

